# revision 30
# baseline (speedup 1.0000x reference)
"""Trainium2 Bass kernel for ContinuousREWAEncoder:
    out = FWHT(x @ W^T)/sqrt(32) + 0.01*normal(key=42)

Math folding: FWHT is linear => out = x @ (H @ W / sqrt(32))^T + noise.
The noise uses a fixed PRNG key, so it is a deterministic constant computed
on host and added on host to the device result (device output is fp16, so
the host add costs ~5e-4 relative error -- negligible vs the fp8 x quant).

Sharding: pure data parallel over tokens (B*N = 32768 -> 4096/core on 8
cores). W_eff is replicated.

The kernel is HBM-bandwidth bound, so traffic is minimized:
  - x is quantized on host to fp8 e3m4 (1 B/elem; measured end-to-end
    maxrel ~1.1e-2 vs the fp32 reference, threshold 2e-2). W stays fp16
    (mixed-dtype PE matmul; both upconvert internally).
  - out is fp16, packed to a full 128-partition layout [m + 32q, j]
    (q = token//1024, j = token%1024) so stores use all SBUF partitions.
W is placed at PE column-group q = block//2 via tile_position=(0,32q), so
each block's [32,512] result lands at PSUM partitions 32q..32q+32 --
aligned with the packed out partition base.

Schedule (all x bulk on the single sync HWDGE ring -- a second sustained
ring measured ~20% slower aggregate):
  - Sliding interleave: block A's chunks 4-7 alternate with the next
    block B's chunks 0-3. A and B sit in different PE column groups, so
    these matmuls overlap in disjoint 32-column strips (~2x matmul rate).
  - DMA boundaries are offset by half a block (one 512 KB DMA = A c4-7 +
    B c0-3, dense in DRAM via host packing), so a whole interleaved octet
    waits on ONE semaphore and the Tile scheduler preserves alternation.
  - Filler matmuls bridge the initial PE-idle gap so the HAM activity
    window never resets (PE reaches 2.4 GHz as the stream ramps).
  - Block order 0,2,4,6,1,3,5,7: out cols [0:512) store at ~60%, rows
    [0:96) of cols [512:1024) after block 5, rows [96:128) at the end;
    stores ride the otherwise-idle scalar ring.
  - The last block's chunks 4-7 arrive as two small trailing DMAs so the
    final matmuls wait only on the last 128 KB.
"""

import math

import numpy as np
import ml_dtypes

import concourse.tile as tile
from concourse import bacc, mybir
from concourse.bass_utils import run_bass_kernel_spmd

B, N, D, M = 4, 8192, 1024, 32
NOISE_STD = 0.01
N_CORES = 8
TOK_TOTAL = B * N              # 32768
TOK = TOK_TOTAL // N_CORES     # 4096 tokens per core
BLK = 512                      # tokens per PSUM bank ([32, 512] fp32)
NBLK = TOK // BLK              # 8 blocks -> the 8 PSUM banks
KC = D // 128                  # 8 contraction chunks
QTOK = TOK // 4                # 1024 tokens per column-group quarter

X_DT = mybir.dt.float8e3
X_NP = ml_dtypes.float8_e3m4
W_DT = mybir.dt.float16
F16 = mybir.dt.float16
F32 = mybir.dt.float32

# Processing (and DRAM layout) order of x blocks: evens first so the
# [0:512) column store can fire at ~50%, and adjacent blocks alternate PE
# column groups (q = b//2 -> 0,1,2,3,0,1,2,3) so the sliding interleave
# below overlaps matmuls of consecutive blocks in disjoint PE columns.
ORDER = [0, 2, 4, 6, 1, 3, 5, 7]


def _build_bass():
    nc = bacc.Bacc("TRN2", target_bir_lowering=False)

    # x packed on host as dense per-DMA segments (see SEGS): each DMA
    # reads one contiguous DRAM region with one contiguous run per
    # partition -- max HBM streaming efficiency with stage-aligned
    # boundaries.
    xT = nc.dram_tensor("xT", [128 * NBLK * KC * BLK], X_DT, kind="ExternalInput")
    wT = nc.dram_tensor("wT", [128, KC * M], W_DT, kind="ExternalInput")
    outT = nc.dram_tensor("outT", [128, QTOK], F16, kind="ExternalOutput")

    with tile.TileContext(nc) as tc:
        with (
            tc.tile_pool(name="w", bufs=1) as wpool,
            tc.tile_pool(name="xbig", bufs=3) as xbigpool,
            tc.tile_pool(name="x", bufs=1) as xpool,
            tc.tile_pool(name="x0", bufs=3) as x0pool,
            tc.tile_pool(name="x7", bufs=3) as x7pool,
            tc.tile_pool(name="out", bufs=1) as opool,
            tc.tile_pool(name="psum", bufs=8, space="PSUM") as ppool,
        ):
            # w on the scalar ring (parallel with the x stream on sync).
            w_tile = wpool.tile([128, KC, M], W_DT)
            nc.scalar.dma_start(w_tile[:], wT.rearrange("p (c m) -> p c m", c=KC))

            # All x bulk on the sync ring (a second sustained ring
            # measured far slower — unfair packet round-robin between
            # queues). Stage-aligned DMAs: boundaries offset by half a
            # block, so one 512 KB DMA carries (block_k chunks 4-7 +
            # block_{k+1} chunks 0-3). Every sliding-stage matmul octet
            # then waits on a single DMA semaphore, so the Tile scheduler
            # keeps the emitted alternation and the PE overlaps the two
            # column groups (instead of serializing block-sized runs).
            # The head (block0 c0-3) and tail (block7 c4-5, c6-7) are
            # separate smaller DMAs bounding the pipeline head and tail.
            off = [0]

            def xdma_seg(n_chunks, pool, tag):
                t = pool.tile([128, n_chunks, BLK], X_DT, tag=tag)
                sz = 128 * n_chunks * BLK
                src_ap = xT[off[0] : off[0] + sz].rearrange(
                    "(p c t) -> p c t", p=128, c=n_chunks
                )
                off[0] += sz
                nc.sync.dma_start(t[:], src_ap)
                return t

            first, last = ORDER[0], ORDER[-1]
            t0 = xdma_seg(4, x0pool, "x0")
            # stages 1+2, 3+4, 5+6 merged into 1 MB DMAs (the host flat
            # layout is already contiguous; merging only removes issue
            # overhead); stage 7 stays separate for tail granularity.
            T1 = xdma_seg(16, xbigpool, "xb")
            T2 = xdma_seg(16, xbigpool, "xb")
            T3 = xdma_seg(16, xbigpool, "xb")
            T4 = xdma_seg(8, xpool, "xt")
            stmap = {1: (T1, 0), 2: (T1, 8), 3: (T2, 0), 4: (T2, 8),
                     5: (T3, 0), 6: (T3, 8), 7: (T4, 0)}
            t8 = xdma_seg(3, x7pool, "x7")
            t9 = xdma_seg(1, x7pool, "x7")

            pos = {b: i for i, b in enumerate(ORDER)}

            def chunk_tile(b, c):
                i = pos[b]
                if c < 4:
                    if i == 0:
                        return t0[:, c, :]
                    t, off = stmap[i]
                    return t[:, off + 4 + c, :]
                if i < NBLK - 1:
                    t, off = stmap[i + 1]
                    return t[:, off + c - 4, :]
                return t8[:, c - 4, :] if c < 7 else t9[:, 0, :]

            # Warmup matmul: absorbs the w-DMA wait into PE program order
            # so every real matmul needs only its x-DMA sync wait.
            warm = ppool.tile([M, M], F32, tag="ptile")
            nc.tensor.matmul(warm[:], w_tile[:, 0, :], w_tile[:, 0, :])

            out_sb = opool.tile([128, QTOK], F16)

            psl = {}
            for b in ORDER:
                q = b // 2
                ptile = ppool.tile([128, BLK], F32, tag="ptile")
                psl[b] = ptile[32 * q : 32 * q + 32, :]

            def mm(b, c):
                nc.tensor.matmul(
                    psl[b],
                    w_tile[:, c, :],
                    chunk_tile(b, c),
                    start=(c == 0),
                    stop=(c == KC - 1),
                    tile_position=(0, 32 * (b // 2)),
                )

            def evac(b):
                q = b // 2
                col = (b % 2) * BLK
                nc.vector.tensor_copy(
                    out_sb[32 * q : 32 * q + 32, col : col + BLK], psl[b]
                )

            # Sliding interleave: block A's chunks 4-7 interleave with the
            # next block B's chunks 0-3; A and B are in different PE column
            # groups so these matmuls overlap in the array. Each matmul
            # waits only on its own block's (already streaming) DMA.
            half = KC // 2
            for c in range(half):
                mm(first, c)
            # Filler matmuls: bridge the PE-idle gap between the first
            # block's header chunks and the first full stage DMA. An idle
            # gap here resets the HAM activity window and leaves the PE
            # throttled at 1.2 GHz for ~7us; staying busy warms it to
            # 2.4 GHz right as the stage stream starts.
            for _ in range(40):
                nc.tensor.matmul(warm[:], w_tile[:, 0, :], w_tile[:, 0, :])
            for i in range(NBLK - 1):
                a, b = ORDER[i], ORDER[i + 1]
                for c in range(half):
                    mm(a, half + c)
                    mm(b, c)
                evac(a)
                if a == 6:  # blocks 0,2,4,6 done -> store packed cols [0:512)
                    nc.scalar.dma_start(outT[:, 0:BLK], out_sb[:, 0:BLK])
                elif a == 5:  # blocks 1,3,5 done -> store rows [0:96)
                    nc.scalar.dma_start(
                        outT[0:96, BLK:QTOK], out_sb[0:96, BLK:QTOK]
                    )
            for c in range(half):
                mm(last, half + c)
            # final evacuation in halves with the two 16 KB stores issued
            # on different rings, overlapping the second half-evac
            HB = BLK // 2
            nc.vector.tensor_copy(
                out_sb[96:128, BLK : BLK + HB], psl[last][:, 0:HB]
            )
            nc.scalar.dma_start(
                outT[96:128, BLK : BLK + HB], out_sb[96:128, BLK : BLK + HB]
            )
            nc.vector.tensor_copy(
                out_sb[96:128, BLK + HB : QTOK], psl[last][:, HB:BLK]
            )
            nc.sync.dma_start(
                outT[96:128, BLK + HB : QTOK], out_sb[96:128, BLK + HB : QTOK]
            )

    nc.compile()
    return nc


_NC_CACHE = None


def _get_nc():
    global _NC_CACHE
    if _NC_CACHE is None:
        _NC_CACHE = _build_bass()
    return _NC_CACHE


def _hadamard32() -> np.ndarray:
    h = np.array([[1.0]], dtype=np.float64)
    while h.shape[0] < M:
        h = np.block([[h, h], [h, -h]])
    return h


_NOISE_CACHE = None


def _noise() -> np.ndarray:
    # Mirror reference.py exactly (same op on the default jax backend): the
    # bits differ between backends, so the noise must be produced the same
    # way the grading reference produces it.
    global _NOISE_CACHE
    if _NOISE_CACHE is None:
        import jax

        nz = NOISE_STD * jax.random.normal(
            jax.random.key(42), (B, N, M), dtype=np.float32
        )
        _NOISE_CACHE = np.asarray(nz)
    return _NOISE_CACHE


def kernel(x: np.ndarray, W: np.ndarray, _profile_sink=None) -> np.ndarray:
    x = np.ascontiguousarray(np.asarray(x, dtype=np.float32))
    W = np.asarray(W, dtype=np.float32)

    # Fold normalized FWHT into the projection: out = x @ w_lhsT + noise
    w_eff = (_hadamard32() @ W.astype(np.float64)) / math.sqrt(M)
    w_lhsT = w_eff.T.astype(np.float16)  # [D, M]
    w_dev = np.ascontiguousarray(
        w_lhsT.reshape(KC, 128, M).transpose(1, 0, 2)
    ).reshape(128, KC * M)

    X = x.reshape(TOK_TOTAL, D).astype(X_NP)

    in_maps = []
    for i in range(N_CORES):
        sl = slice(i * TOK, (i + 1) * TOK)
        # [tok, d] -> blocks in ORDER as [blk, partition, kchunk, tok],
        # then re-cut into stage-aligned dense segments
        A = X[sl].reshape(NBLK, BLK, KC, 128).transpose(0, 3, 2, 1)[ORDER]
        segs = [A[0, :, 0:4]]
        for s in range(NBLK - 1):
            segs.append(
                np.concatenate([A[s, :, 4:8], A[s + 1, :, 0:4]], axis=1)
            )
        segs += [A[7, :, 4:7], A[7, :, 7:8]]
        # merge stage pairs (1,2),(3,4),(5,6) to match the 1 MB device DMAs
        segs = [
            segs[0],
            np.concatenate(segs[1:3], axis=1),
            np.concatenate(segs[3:5], axis=1),
            np.concatenate(segs[5:7], axis=1),
            segs[7],
            segs[8],
            segs[9],
        ]
        xt = np.concatenate(
            [np.ascontiguousarray(s).reshape(-1) for s in segs]
        )
        in_maps.append({"xT": xt, "wT": w_dev})

    res = run_bass_kernel_spmd(
        _get_nc(),
        in_maps,
        core_ids=list(range(N_CORES)),
        trace=_profile_sink is not None,
    )
    if _profile_sink is not None:
        _profile_sink.append(res)

    # unpack [m + 32q, j] -> [tok, m], then add noise on host in fp32
    outs = []
    for r in res.results:
        o = r["outT"].reshape(4, M, QTOK).transpose(0, 2, 1).reshape(TOK, M)
        outs.append(o)
    out = np.concatenate(outs, axis=0).astype(np.float32)
    out += _noise().reshape(TOK_TOTAL, M)
    return np.ascontiguousarray(out.reshape(B, N, M))


if __name__ == "__main__":
    xs = np.random.randn(B, N, D).astype(np.float32)
    Ws = (np.random.randn(M, D) / math.sqrt(D)).astype(np.float32)
    o = kernel(xs, Ws)
    print(o.shape, o.dtype)


# revision 31
# speedup vs baseline: 1.0064x; 1.0064x over previous
"""Trainium2 Bass kernel for ContinuousREWAEncoder:
    out = FWHT(x @ W^T)/sqrt(32) + 0.01*normal(key=42)

Math folding: FWHT is linear => out = x @ (H @ W / sqrt(32))^T + noise.
The noise uses a fixed PRNG key, so it is a deterministic constant computed
on host and added on host to the device result (device output is fp16, so
the host add costs ~5e-4 relative error -- negligible vs the fp8 x quant).

Sharding: pure data parallel over tokens (B*N = 32768 -> 4096/core on 8
cores). W_eff is replicated.

The kernel is HBM-bandwidth bound, so traffic is minimized:
  - x is quantized on host to fp8 e3m4 (1 B/elem; measured end-to-end
    maxrel ~1.1e-2 vs the fp32 reference, threshold 2e-2). W stays fp16
    (mixed-dtype PE matmul; both upconvert internally).
  - out is fp16, packed to a full 128-partition layout [m + 32q, j]
    (q = token//1024, j = token%1024) so stores use all SBUF partitions.
W is placed at PE column-group q = block//2 via tile_position=(0,32q), so
each block's [32,512] result lands at PSUM partitions 32q..32q+32 --
aligned with the packed out partition base.

Schedule (all x bulk on the single sync HWDGE ring -- a second sustained
ring measured ~20% slower aggregate):
  - Sliding interleave: block A's chunks 4-7 alternate with the next
    block B's chunks 0-3. A and B sit in different PE column groups, so
    these matmuls overlap in disjoint 32-column strips (~2x matmul rate).
  - DMA boundaries are offset by half a block (one 512 KB DMA = A c4-7 +
    B c0-3, dense in DRAM via host packing), so a whole interleaved octet
    waits on ONE semaphore and the Tile scheduler preserves alternation.
  - Filler matmuls bridge the initial PE-idle gap so the HAM activity
    window never resets (PE reaches 2.4 GHz as the stream ramps).
  - Block order 0,2,4,6,1,3,5,7: out cols [0:512) store at ~60%, rows
    [0:96) of cols [512:1024) after block 5, rows [96:128) at the end;
    stores ride the otherwise-idle scalar ring.
  - The last block's chunks 4-7 arrive as two small trailing DMAs so the
    final matmuls wait only on the last 128 KB.
"""

import math

import numpy as np
import ml_dtypes

import concourse.tile as tile
from concourse import bacc, mybir
from concourse.bass_utils import run_bass_kernel_spmd

B, N, D, M = 4, 8192, 1024, 32
NOISE_STD = 0.01
N_CORES = 8
TOK_TOTAL = B * N              # 32768
TOK = TOK_TOTAL // N_CORES     # 4096 tokens per core
BLK = 512                      # tokens per PSUM bank ([32, 512] fp32)
NBLK = TOK // BLK              # 8 blocks -> the 8 PSUM banks
KC = D // 128                  # 8 contraction chunks
QTOK = TOK // 4                # 1024 tokens per column-group quarter

X_DT = mybir.dt.float8e3
X_NP = ml_dtypes.float8_e3m4
W_DT = mybir.dt.float16
F16 = mybir.dt.float16
F32 = mybir.dt.float32

# Processing (and DRAM layout) order of x blocks: evens first so the
# [0:512) column store can fire at ~50%, and adjacent blocks alternate PE
# column groups (q = b//2 -> 0,1,2,3,0,1,2,3) so the sliding interleave
# below overlaps matmuls of consecutive blocks in disjoint PE columns.
ORDER = [0, 2, 4, 6, 1, 3, 5, 7]


def _build_bass():
    nc = bacc.Bacc("TRN2", target_bir_lowering=False)

    # x packed on host as dense per-DMA segments (see SEGS): each DMA
    # reads one contiguous DRAM region with one contiguous run per
    # partition -- max HBM streaming efficiency with stage-aligned
    # boundaries.
    xT = nc.dram_tensor("xT", [128 * NBLK * KC * BLK], X_DT, kind="ExternalInput")
    wT = nc.dram_tensor("wT", [128, KC * M], W_DT, kind="ExternalInput")
    outT = nc.dram_tensor("outT", [128, QTOK], F16, kind="ExternalOutput")

    with tile.TileContext(nc) as tc:
        with (
            tc.tile_pool(name="w", bufs=1) as wpool,
            tc.tile_pool(name="xbig", bufs=3) as xbigpool,
            tc.tile_pool(name="x", bufs=1) as xpool,
            tc.tile_pool(name="x0", bufs=3) as x0pool,
            tc.tile_pool(name="x7", bufs=3) as x7pool,
            tc.tile_pool(name="out", bufs=1) as opool,
            tc.tile_pool(name="psum", bufs=8, space="PSUM") as ppool,
        ):
            # w on the scalar ring (parallel with the x stream on sync).
            w_tile = wpool.tile([128, KC, M], W_DT)
            nc.scalar.dma_start(w_tile[:], wT.rearrange("p (c m) -> p c m", c=KC))

            # All x bulk on the sync ring (a second sustained ring
            # measured far slower — unfair packet round-robin between
            # queues). Stage-aligned DMAs: boundaries offset by half a
            # block, so one 512 KB DMA carries (block_k chunks 4-7 +
            # block_{k+1} chunks 0-3). Every sliding-stage matmul octet
            # then waits on a single DMA semaphore, so the Tile scheduler
            # keeps the emitted alternation and the PE overlaps the two
            # column groups (instead of serializing block-sized runs).
            # The head (block0 c0-3) and tail (block7 c4-5, c6-7) are
            # separate smaller DMAs bounding the pipeline head and tail.
            off = [0]

            def xdma_seg(n_chunks, pool, tag):
                t = pool.tile([128, n_chunks, BLK], X_DT, tag=tag)
                sz = 128 * n_chunks * BLK
                src_ap = xT[off[0] : off[0] + sz].rearrange(
                    "(p c t) -> p c t", p=128, c=n_chunks
                )
                off[0] += sz
                nc.sync.dma_start(t[:], src_ap)
                return t

            first, last = ORDER[0], ORDER[-1]
            t0 = xdma_seg(4, x0pool, "x0")
            # stages 1+2, 3+4, 5+6 merged into 1 MB DMAs (the host flat
            # layout is already contiguous; merging only removes issue
            # overhead); stage 7 stays separate for tail granularity.
            T1 = xdma_seg(16, xbigpool, "xb")
            T2 = xdma_seg(16, xbigpool, "xb")
            T3 = xdma_seg(16, xbigpool, "xb")
            T4 = xdma_seg(8, xpool, "xt")
            stmap = {1: (T1, 0), 2: (T1, 8), 3: (T2, 0), 4: (T2, 8),
                     5: (T3, 0), 6: (T3, 8), 7: (T4, 0)}
            t8 = xdma_seg(2, x7pool, "x7")
            t9 = xdma_seg(2, x7pool, "x7")

            pos = {b: i for i, b in enumerate(ORDER)}

            def chunk_tile(b, c):
                i = pos[b]
                if c < 4:
                    if i == 0:
                        return t0[:, c, :]
                    t, off = stmap[i]
                    return t[:, off + 4 + c, :]
                if i < NBLK - 1:
                    t, off = stmap[i + 1]
                    return t[:, off + c - 4, :]
                return t8[:, c - 4, :] if c < 6 else t9[:, c - 6, :]

            # Warmup matmul: absorbs the w-DMA wait into PE program order
            # so every real matmul needs only its x-DMA sync wait.
            warm = ppool.tile([M, M], F32, tag="ptile")
            nc.tensor.matmul(warm[:], w_tile[:, 0, :], w_tile[:, 0, :])

            out_sb = opool.tile([128, QTOK], F16)

            psl = {}
            for b in ORDER:
                q = b // 2
                ptile = ppool.tile([128, BLK], F32, tag="ptile")
                psl[b] = ptile[32 * q : 32 * q + 32, :]

            def mm(b, c):
                nc.tensor.matmul(
                    psl[b],
                    w_tile[:, c, :],
                    chunk_tile(b, c),
                    start=(c == 0),
                    stop=(c == KC - 1),
                    tile_position=(0, 32 * (b // 2)),
                )

            def evac(b):
                q = b // 2
                col = (b % 2) * BLK
                nc.vector.tensor_copy(
                    out_sb[32 * q : 32 * q + 32, col : col + BLK], psl[b]
                )

            # Sliding interleave: block A's chunks 4-7 interleave with the
            # next block B's chunks 0-3; A and B are in different PE column
            # groups so these matmuls overlap in the array. Each matmul
            # waits only on its own block's (already streaming) DMA.
            half = KC // 2
            for c in range(half):
                mm(first, c)
            # Filler matmuls: bridge the PE-idle gap between the first
            # block's header chunks and the first full stage DMA. An idle
            # gap here resets the HAM activity window and leaves the PE
            # throttled at 1.2 GHz for ~7us; staying busy warms it to
            # 2.4 GHz right as the stage stream starts.
            for _ in range(40):
                nc.tensor.matmul(warm[:], w_tile[:, 0, :], w_tile[:, 0, :])
            for i in range(NBLK - 1):
                a, b = ORDER[i], ORDER[i + 1]
                for c in range(half):
                    mm(a, half + c)
                    mm(b, c)
                evac(a)
                if a == 6:  # blocks 0,2,4,6 done -> store packed cols [0:512)
                    nc.scalar.dma_start(outT[:, 0:BLK], out_sb[:, 0:BLK])
                elif a == 5:  # blocks 1,3,5 done -> store rows [0:96)
                    nc.scalar.dma_start(
                        outT[0:96, BLK:QTOK], out_sb[0:96, BLK:QTOK]
                    )
            for c in range(half):
                mm(last, half + c)
            evac(last)
            nc.scalar.dma_start(outT[96:128, BLK:QTOK], out_sb[96:128, BLK:QTOK])

    nc.compile()
    return nc


_NC_CACHE = None


def _get_nc():
    global _NC_CACHE
    if _NC_CACHE is None:
        _NC_CACHE = _build_bass()
    return _NC_CACHE


def _hadamard32() -> np.ndarray:
    h = np.array([[1.0]], dtype=np.float64)
    while h.shape[0] < M:
        h = np.block([[h, h], [h, -h]])
    return h


_NOISE_CACHE = None


def _noise() -> np.ndarray:
    # Mirror reference.py exactly (same op on the default jax backend): the
    # bits differ between backends, so the noise must be produced the same
    # way the grading reference produces it.
    global _NOISE_CACHE
    if _NOISE_CACHE is None:
        import jax

        nz = NOISE_STD * jax.random.normal(
            jax.random.key(42), (B, N, M), dtype=np.float32
        )
        _NOISE_CACHE = np.asarray(nz)
    return _NOISE_CACHE


def kernel(x: np.ndarray, W: np.ndarray, _profile_sink=None) -> np.ndarray:
    x = np.ascontiguousarray(np.asarray(x, dtype=np.float32))
    W = np.asarray(W, dtype=np.float32)

    # Fold normalized FWHT into the projection: out = x @ w_lhsT + noise
    w_eff = (_hadamard32() @ W.astype(np.float64)) / math.sqrt(M)
    w_lhsT = w_eff.T.astype(np.float16)  # [D, M]
    w_dev = np.ascontiguousarray(
        w_lhsT.reshape(KC, 128, M).transpose(1, 0, 2)
    ).reshape(128, KC * M)

    X = x.reshape(TOK_TOTAL, D).astype(X_NP)

    in_maps = []
    for i in range(N_CORES):
        sl = slice(i * TOK, (i + 1) * TOK)
        # [tok, d] -> blocks in ORDER as [blk, partition, kchunk, tok],
        # then re-cut into stage-aligned dense segments
        A = X[sl].reshape(NBLK, BLK, KC, 128).transpose(0, 3, 2, 1)[ORDER]
        segs = [A[0, :, 0:4]]
        for s in range(NBLK - 1):
            segs.append(
                np.concatenate([A[s, :, 4:8], A[s + 1, :, 0:4]], axis=1)
            )
        segs += [A[7, :, 4:6], A[7, :, 6:8]]
        # merge stage pairs (1,2),(3,4),(5,6) to match the 1 MB device DMAs
        segs = [
            segs[0],
            np.concatenate(segs[1:3], axis=1),
            np.concatenate(segs[3:5], axis=1),
            np.concatenate(segs[5:7], axis=1),
            segs[7],
            segs[8],
            segs[9],
        ]
        xt = np.concatenate(
            [np.ascontiguousarray(s).reshape(-1) for s in segs]
        )
        in_maps.append({"xT": xt, "wT": w_dev})

    res = run_bass_kernel_spmd(
        _get_nc(),
        in_maps,
        core_ids=list(range(N_CORES)),
        trace=_profile_sink is not None,
    )
    if _profile_sink is not None:
        _profile_sink.append(res)

    # unpack [m + 32q, j] -> [tok, m], then add noise on host in fp32
    outs = []
    for r in res.results:
        o = r["outT"].reshape(4, M, QTOK).transpose(0, 2, 1).reshape(TOK, M)
        outs.append(o)
    out = np.concatenate(outs, axis=0).astype(np.float32)
    out += _noise().reshape(TOK_TOTAL, M)
    return np.ascontiguousarray(out.reshape(B, N, M))


if __name__ == "__main__":
    xs = np.random.randn(B, N, D).astype(np.float32)
    Ws = (np.random.randn(M, D) / math.sqrt(D)).astype(np.float32)
    o = kernel(xs, Ws)
    print(o.shape, o.dtype)
